# revision 1
# baseline (speedup 1.0000x reference)
"""MoE top-2 routing kernel for 8 Trainium2 NeuronCores.

Problem: x[2,4096,1024] tokens, 8 experts W[8,1024,1024]+b[8,1024],
top-2 expert indices + gate weights per token.
out[t] = sum_k gate[t,k] * (x[t] @ W[idx[t,k]] + b[idx[t,k]])

Strategy (data-parallel dispatch):
- Flatten tokens to [8192, 1024]; core c owns tokens [c*1024,(c+1)*1024).
- Host computes routing from the (input) indices: per expert, the list of
  (local token, gate), same-expert duplicates merged (gates summed), each
  expert segment padded to a multiple of 128 rows (pad = token 0, gate 0).
  Per-expert tile capacities are maxed across cores so all 8 cores run one
  SPMD program.
- On-chip per expert: dma_gather(transpose=True) pulls the routed token rows
  of x (fp16) from DRAM directly into the transposed [128d x ntok] layout the
  PE needs; 128-token tiles are matmul'd against W_e (fp16, f32 PSUM
  accumulation over 8 K-chunks) with the bias added via a ones-row matmul;
  DVE scales rows by the gate; dma_scatter_add accumulates rows into the
  zero-initialized f32 output.
- fp16 keeps absmax error ~3e-4 of output scale (vs 2e-3 for bf16) at
  identical PE throughput.
"""

import os
import sys

import numpy as np

for _p in ("/opt/trn_rl_repo", os.path.expanduser("~/.axon_site/_ro/trn_rl_repo")):
    if os.path.isdir(_p) and _p not in sys.path:
        sys.path.insert(0, _p)

B, S, D, E, K = 2, 4096, 1024, 8, 2
N_CORES = 8
TOKENS = B * S
TOK_PER_CORE = TOKENS // N_CORES  # 1024
P = 128
DCHUNKS = D // P  # 8
FH = 512  # psum bank half of D
NH = D // FH  # 2


def _build_routing(top_k_indices, expert_weights):
    """Balanced sharding + routing.

    Token->core assignment is part of the sharding strategy: a greedy
    balancer equalizes per-(core, expert) entry counts so every expert fits
    the same (minimal) per-core tile capacity on all 8 cores — the SPMD
    program pads each expert segment to caps[e]*128 rows.

    Returns (tokens_per_core, per_core_lists, caps):
      tokens_per_core[c]: global token ids assigned to core c (len 1024)
      per_core_lists[c][e]: [(local_token_pos, gate), ...] deduped
      caps[e]: tiles for expert e (same on every core)
    """
    idx = np.asarray(top_k_indices).reshape(-1, K)
    gw = np.asarray(expert_weights).reshape(-1, K).astype(np.float32)
    n_tok = idx.shape[0]
    entries = []
    for t in range(n_tok):
        e0, e1 = int(idx[t, 0]), int(idx[t, 1])
        g0, g1 = float(gw[t, 0]), float(gw[t, 1])
        if e0 == e1:
            entries.append([(e0, g0 + g1)])
        else:
            entries.append([(e0, g0), (e1, g1)])
    count = [[0] * E for _ in range(N_CORES)]
    n = [0] * N_CORES
    assign = [0] * n_tok
    for t in sorted(range(n_tok), key=lambda t: -len(entries[t])):
        exps = [e for e, _ in entries[t]]
        best, bs = -1, None
        for c in range(N_CORES):
            if n[c] >= TOK_PER_CORE:
                continue
            s = (
                max(count[c][e] for e in exps),
                sum(count[c][e] for e in exps),
                n[c],
            )
            if bs is None or s < bs:
                best, bs = c, s
        assign[t] = best
        n[best] += 1
        for e in exps:
            count[best][e] += 1
    tokens_per_core = [
        [t for t in range(n_tok) if assign[t] == c] for c in range(N_CORES)
    ]
    # Tile caps per expert (same on every core) and the expert processing
    # order (largest first → smallest last, so the kernel tail is minimal).
    caps = [0] * E
    for c in range(N_CORES):
        cnt = [0] * E
        for t in tokens_per_core[c]:
            for e, _ in entries[t]:
                cnt[e] += 1
        for e in range(E):
            caps[e] = max(caps[e], max(1, (cnt[e] + P - 1) // P))
    order = sorted(range(E), key=lambda e: -caps[e])

    # Reorder each core's tokens so that the FIRST processed expert's
    # segment tokens sit at local rows 0..n0-1 in segment order: its
    # "gather" is then the identity and the kernel loads it with plain
    # HWDGE DMA-transpose (available ~15 µs before the Q7 gather ucode).
    e0 = order[0]
    per_core = []
    for c in range(N_CORES):
        first = [t for t in tokens_per_core[c] if any(e == e0 for e, _ in entries[t])]
        rest = [t for t in tokens_per_core[c] if not any(e == e0 for e, _ in entries[t])]
        toks = first + rest
        tokens_per_core[c] = np.asarray(toks, np.int64)
        lists = [[] for _ in range(E)]
        for local, t in enumerate(toks):
            for e, g in entries[t]:
                lists[e].append((local, g))
        per_core.append(lists)
        assert [p for p, _ in lists[e0]] == list(range(len(lists[e0])))
    return tokens_per_core, per_core, caps, order


def _wrap_idxs(idx_disp):
    # dma_gather/scatter idx layout: logical i lives at [i % 16, i // 16],
    # replicated into all eight 16-partition groups (each GpSimd Q7 core
    # reads the group at [16*cpu_id, 16*cpu_id + 16)).
    tot = idx_disp.size
    return np.tile(idx_disp.reshape(tot // 16, 16).T, (P // 16, 1))


def _build_dispatch(lists, caps, order):
    """Gather/scatter index (int16) and gate (f32) arrays, segments laid out
    in expert PROCESSING order (so consecutive experts are contiguous)."""
    gidx_disp = []  # gather: dummy rows read x row 0 (real data, gate 0)
    sidx_disp = []  # scatter: dummy rows add their (zero) output to a trash row
    gate_disp = []
    for e in order:
        n = caps[e] * P
        ent = lists[e]
        assert len(ent) <= n
        pad = n - len(ent)
        gidx_disp += [t for t, _ in ent] + [0] * pad
        sidx_disp += [t for t, _ in ent] + [TOK_PER_CORE] * pad
        gate_disp += [g for _, g in ent] + [0.0] * pad
    gidx_disp = np.asarray(gidx_disp, np.int16)
    sidx_disp = np.asarray(sidx_disp, np.int16)
    gate_disp = np.asarray(gate_disp, np.float32)
    tot = gate_disp.size
    # gate layout: dispatch row i at [i % 128, i // 128]
    gates_sb = np.ascontiguousarray(gate_disp.reshape(tot // P, P).T)
    return _wrap_idxs(gidx_disp), _wrap_idxs(sidx_disp), gates_sb


def _build_program(caps, order):
    import concourse.bass as bass  # noqa: F401
    import concourse.tile as tile
    from concourse import bacc, mybir

    fp16 = mybir.dt.float16
    f32 = mybir.dt.float32
    i16 = mybir.dt.int16

    tot = sum(caps) * P
    nc = bacc.Bacc("TRN2", target_bir_lowering=False, debug=False)

    x_d = nc.dram_tensor("x", [TOK_PER_CORE, D], fp16, kind="ExternalInput").ap()
    w_d = nc.dram_tensor("w", [E * P, DCHUNKS * D], fp16, kind="ExternalInput").ap()
    b_d = nc.dram_tensor("b", [1, E * D], fp16, kind="ExternalInput").ap()
    idx_d = nc.dram_tensor("idxs", [P, tot // 16], i16, kind="ExternalInput").ap()
    sidx_d = nc.dram_tensor("sidxs", [P, tot // 16], i16, kind="ExternalInput").ap()
    gate_d = nc.dram_tensor("gates", [P, tot // P], f32, kind="ExternalInput").ap()
    # fp16 output: scatter_add is read-modify-write traffic, so halving the
    # element size halves the heaviest stream (host upcasts to f32).
    # +8 trash rows: dummy (padding) scatter rows accumulate their exact-zero
    # payload there instead of racing real rows.
    out_d = nc.dram_tensor(
        "out", [TOK_PER_CORE + 8, D], fp16, kind="ExternalOutput"
    ).ap()

    # Dispatch segments are laid out in processing order; off[k] is the tile
    # offset of the k-th processed expert.
    off = [sum(caps[order[j]] for j in range(k)) for k in range(E)]

    with tile.TileContext(nc) as tc:
        with (
            tc.tile_pool(name="const", bufs=1) as cpool,
            tc.tile_pool(name="wpool", bufs=3) as wpool,
            tc.tile_pool(name="xgpool", bufs=5) as xgpool,
            tc.tile_pool(name="ypool", bufs=6) as ypool,
            tc.tile_pool(name="pspool", bufs=2, space="PSUM") as pspool,
        ):
            # Tiny metadata loads first (the first gather needs idx_sb).
            idx_sb = cpool.tile([P, tot // 16], i16)
            nc.sync.dma_start(idx_sb[:], idx_d[:])
            sidx_sb = cpool.tile([P, tot // 16], i16)
            nc.scalar.dma_start(sidx_sb[:], sidx_d[:])
            gate_sb = cpool.tile([P, tot // P], f32)
            nc.scalar.dma_start(gate_sb[:], gate_d[:])
            b_sb = cpool.tile([1, E * D], fp16)
            nc.scalar.dma_start(b_sb[:], b_d[:])
            ones_sb = cpool.tile([1, P], fp16)
            nc.vector.memset(ones_sb[:], 1.0)

            # Q7 ucode warmup: the gather/scatter kernels live in a
            # tensor-delivered library the Q7 loads on first use (~10 µs).
            # Pay that cost now, overlapped with the W/idx loads.
            warm_idx = cpool.tile([P, 8], i16)
            nc.gpsimd.memset(warm_idx[:], 0)
            warm_out = cpool.tile([P, 1, P], fp16)
            nc.gpsimd.dma_gather(
                warm_out[:],
                x_d[:, 0:P],
                warm_idx[:],
                P,
                P,
                P,
                elem_step=D,
                transpose=True,
            )

            # Software-pipelined prefetch: W on the HWDGE/sync ring (depth 2),
            # the x row-gathers on SWDGE (depth 4 — gather preps must stay
            # well ahead of the scatter epilogue-waits in the gpsimd stream).
            pref_w = {}
            pref_x = {}

            def prefetch_w(k):
                if k >= E:
                    return
                e = order[k]
                w_sb = wpool.tile([P, DCHUNKS, D], fp16, tag="w", name="w_sb")
                nc.sync.dma_start(
                    w_sb[:],
                    w_d[e * P : (e + 1) * P, :].rearrange(
                        "p (c d) -> p c d", c=DCHUNKS
                    ),
                )
                pref_w[k] = w_sb

            def prefetch_x(k):
                if k >= E:
                    return
                e = order[k]
                n_e = caps[e] * P
                col0 = off[k] * (P // 16)
                if k == 0:
                    # First expert: per-tile gathers so the first matmuls can
                    # start as soon as tile 0's rows land (the big upfront
                    # burst otherwise delays the first data by ~7 µs).
                    xgs = []
                    for t in range(caps[e]):
                        xg0 = xgpool.tile([P, DCHUNKS, P], fp16, tag="xg0", name="xg0")
                        colt = (off[k] + t) * (P // 16)
                        nc.gpsimd.dma_gather(
                            xg0[:],
                            x_d[:],
                            idx_sb[:, colt : colt + P // 16],
                            P,
                            P,
                            D,
                            transpose=True,
                        )
                        xgs.append(xg0)
                    pref_x[k] = xgs
                    return
                xg = xgpool.tile([P, DCHUNKS, n_e], fp16, tag="xg", name="xg")
                nc.gpsimd.dma_gather(
                    xg[:],
                    x_d[:],
                    idx_sb[:, col0 : col0 + n_e // 16],
                    n_e,
                    n_e,
                    D,
                    transpose=True,
                )
                pref_x[k] = xg

            prefetch_x(0)
            for _k in range(2):
                prefetch_w(_k)
            for _k in range(1, 4):
                prefetch_x(_k)

            # PE warmup: ~5 µs of dummy matmuls while the first gather/W
            # loads run, so the HAM clock-gate reaches 2.4 GHz before the
            # first real matmul.
            warm_ps = pspool.tile([P, P], f32, tag="warm")
            for _ in range(140):
                nc.tensor.matmul(
                    warm_ps[:], ones_sb[0:1, :], ones_sb[0:1, :], start=True, stop=True
                )

            # Zero-init the output (scatter_add accumulates into it) on the
            # scalar HWDGE ring, overlapping the first expert's compute.
            zero_sb = cpool.tile([P, D], fp16)
            nc.vector.memset(zero_sb[:], 0.0)
            for r in range(TOK_PER_CORE // P):
                nc.scalar.dma_start(out_d[r * P : (r + 1) * P, :], zero_sb[:])

            # One scatter per expert: rows within a call are distinct tokens
            # (dedup + trash-row padding), so no RMW races inside a call;
            # Tile's WAW tracking chains the calls against each other.
            for k in range(E):
                e = order[k]
                ce = caps[e]
                n_e = ce * P
                col0 = off[k] * (P // 16)
                w_sb = pref_w.pop(k)
                xg = pref_x.pop(k)
                y_sb = ypool.tile([P, ce, D], fp16, tag="y", name="y_sb")
                for t in range(ce):
                    ps = pspool.tile([P, D], f32, tag="ps", name="ps")
                    for c in range(DCHUNKS + 1):
                        for h in range(NH):
                            if c < DCHUNKS:
                                if k == 0:
                                    lhsT = xg[t][:, c, :]
                                else:
                                    lhsT = xg[:, c, t * P : (t + 1) * P]
                                rhs = w_sb[:, c, h * FH : (h + 1) * FH]
                            else:
                                lhsT = ones_sb[0:1, :]
                                rhs = b_sb[0:1, e * D + h * FH : e * D + (h + 1) * FH]
                            nc.tensor.matmul(
                                ps[:, h * FH : (h + 1) * FH],
                                lhsT,
                                rhs,
                                start=(c == 0),
                                stop=(c == DCHUNKS),
                            )
                    gt = off[k] + t
                    nc.vector.tensor_scalar_mul(
                        y_sb[:, t, :], ps[:, :], gate_sb[:, gt : gt + 1]
                    )
                    if k == 0:
                        # First expert scatters per tile: the (serialized)
                        # scatter chain starts one epilogue earlier.
                        colt = (off[k] + t) * (P // 16)
                        nc.gpsimd.dma_scatter_add(
                            out_d[:],
                            y_sb[:, t : t + 1, :],
                            sidx_sb[:, colt : colt + P // 16],
                            P,
                            P,
                            D,
                        )
                # Gather prefetch BEFORE the scatter's epilogue-wait in the
                # gpsimd stream; W prefetch on the sync ring.
                prefetch_x(k + 4)
                prefetch_w(k + 2)
                if k > 0:
                    nc.gpsimd.dma_scatter_add(
                        out_d[:],
                        y_sb[:],
                        sidx_sb[:, col0 : col0 + n_e // 16],
                        n_e,
                        n_e,
                        D,
                    )
    nc.compile()
    return nc


def _prep_inputs(x, expert_weights, top_k_indices, W, b):
    """Host-side sharding: per-core input maps + caps + token assignment."""
    tokens_per_core, per_core, caps, order = _build_routing(
        top_k_indices, expert_weights
    )
    x_flat = np.asarray(x, np.float32).reshape(TOKENS, D)
    w_hw = np.ascontiguousarray(
        np.asarray(W, np.float32)
        .reshape(E, DCHUNKS, P, D)
        .transpose(0, 2, 1, 3)
        .astype(np.float16)
        .reshape(E * P, DCHUNKS * D)
    )
    b_hw = np.ascontiguousarray(np.asarray(b, np.float32).astype(np.float16).reshape(1, E * D))
    in_maps = []
    for c in range(N_CORES):
        idxs_sb, sidxs_sb, gates_sb = _build_dispatch(per_core[c], caps, order)
        xc = np.ascontiguousarray(x_flat[tokens_per_core[c]].astype(np.float16))
        in_maps.append(
            {
                "x": xc,
                "w": w_hw,
                "b": b_hw,
                "idxs": idxs_sb,
                "sidxs": sidxs_sb,
                "gates": gates_sb,
            }
        )
    return in_maps, caps, order, tokens_per_core


def kernel(x, expert_weights, top_k_indices, W, b):
    from concourse.bass_utils import run_bass_kernel_spmd

    in_maps, caps, order, tokens_per_core = _prep_inputs(
        x, expert_weights, top_k_indices, W, b
    )
    nc = _build_program(caps, order)
    res = run_bass_kernel_spmd(
        nc,
        in_maps,
        core_ids=list(range(N_CORES)),
        trace=bool(int(os.environ.get("KERNEL_TRACE", "0"))),
    )
    out = np.empty((TOKENS, D), np.float32)
    for c in range(N_CORES):
        out[tokens_per_core[c]] = res.results[c]["out"][:TOK_PER_CORE].astype(
            np.float32
        )
    if bool(int(os.environ.get("KERNEL_TRACE", "0"))):
        kernel.last_results = res
    return np.ascontiguousarray(out.reshape(B, S, D))



# revision 4
# speedup vs baseline: 1.6242x; 1.6242x over previous
"""MoE top-2 routing kernel for 8 Trainium2 NeuronCores — expert-parallel.

Problem: x[2,4096,1024] tokens, 8 experts W[8,1024,1024]+b[8,1024],
top-2 expert indices + gate weights per token.
out[t] = sum_k gate[t,k] * (x[t] @ W[idx[t,k]] + b[idx[t,k]])

Strategy (expert-parallel, host-side dispatch):
- E == n_cores == 8: core e owns expert e. The host routes: dedup the two
  (expert, gate) entries per token (same-expert duplicates merge, gates
  summed), groups entries by expert, and builds per-core inputs:
    xg   [128, Tmax*1024] fp16 — the expert's token rows, PE-transposed
         ([p, t*1024 + c*128 + m] = x[row t*128+m, c*128+p]), zero-padded
         to Tmax 128-row tiles.
    w    [128, 8*1024]    fp16 — W_e PE layout ([p, c*1024+f] = W_e[c*128+p, f])
    g    [128, Tmax]      f32  — gate per dispatch row ([m, t] = gate row t*128+m)
- Device: per 128-row tile, 16 accumulating fp16 matmuls (8 K-chunks x 2
  PSUM halves) -> DVE gate-scale (f32 PSUM -> fp16) -> contiguous DMA
  store. No gather/scatter ucode, no replicated W, no bias matmuls.
- Host combine: out[t] = Y[slot0[t]] + Y[slot1[t]] + g0*b[e0] + g1*b[e1]
  (slot1 -> zero row for merged/single-entry tokens); bias exact in f32.
- Load balance: per-expert entry counts are multinomial(~1920 +/- 40);
  Tmax = max_e ceil(n_e/128) == ceil(total_tiles/8) for typical draws, so
  expert-parallel matches the best possible row balance.
"""

import os
import sys

import numpy as np

for _p in ("/opt/trn_rl_repo", os.path.expanduser("~/.axon_site/_ro/trn_rl_repo")):
    if os.path.isdir(_p) and _p not in sys.path:
        sys.path.insert(0, _p)

B, S, D, E, K = 2, 4096, 1024, 8, 2
N_CORES = 8
TOKENS = B * S
P = 128
DCHUNKS = D // P  # 8
FH = 512  # psum bank half of D
NH = D // FH  # 2
WARMUP = 24


def _route(top_k_indices, expert_weights):
    """Dedup + group entries by expert.

    Returns (toks, gs, n_e, Tmax, cum, slot):
      toks/gs: token id and gate per dispatch entry, sorted by expert
      n_e[e]: entry count of expert e; cum[e]: its offset in the sort
      Tmax: per-core tile count = max_e ceil(n_e/128)
      slot[t, 0:2]: global padded-Y row of token t's entries (ZROW = none)
    """
    idx = np.asarray(top_k_indices).reshape(-1, K).astype(np.int64)
    gw = np.asarray(expert_weights).reshape(-1, K).astype(np.float32)
    dup = idx[:, 0] == idx[:, 1]
    g0 = np.where(dup, gw[:, 0] + gw[:, 1], gw[:, 0])
    keep = ~dup
    toks = np.concatenate([np.arange(TOKENS), np.arange(TOKENS)[keep]])
    exps = np.concatenate([idx[:, 0], idx[keep, 1]])
    gs = np.concatenate([g0, gw[keep, 1]])
    order = np.argsort(exps, kind="stable")
    toks, exps, gs = toks[order], exps[order], gs[order]
    n_e = np.bincount(exps, minlength=E)
    Tmax = max(1, int(np.max(-(-n_e // P))))
    cum = np.concatenate([[0], np.cumsum(n_e)])[:E]
    pos_in_e = np.arange(toks.size) - cum[exps]
    yrow = exps * (Tmax * P) + pos_in_e
    inv = np.empty_like(order)
    inv[order] = np.arange(order.size)
    ZROW = E * Tmax * P
    slot = np.full((TOKENS, 2), ZROW, np.int64)
    slot[:, 0] = yrow[inv[:TOKENS]]
    slot[keep, 1] = yrow[inv[TOKENS:]]
    return toks, gs, n_e, Tmax, cum, slot


def _prep_inputs(x, top_k_indices, expert_weights, W):
    toks, gs, n_e, Tmax, cum, slot = _route(top_k_indices, expert_weights)
    x_flat = np.asarray(x, np.float32).reshape(TOKENS, D).astype(np.float16)
    W16 = np.asarray(W, np.float32).astype(np.float16)
    in_maps = []
    for e in range(E):
        n = int(n_e[e])
        seg = slice(cum[e], cum[e] + n)
        xr = np.zeros((Tmax * P, D), np.float16)
        xr[:n] = x_flat[toks[seg]]
        # [t*128+m, c*128+p] -> [p, t, c, m]
        xg = np.ascontiguousarray(
            xr.reshape(Tmax, P, DCHUNKS, P).transpose(3, 0, 2, 1)
        ).reshape(P, Tmax * D)
        gr = np.zeros(Tmax * P, np.float32)
        gr[:n] = gs[seg]
        g_sb = np.ascontiguousarray(gr.reshape(Tmax, P).T)
        w_hw = np.ascontiguousarray(
            W16[e].reshape(DCHUNKS, P, D).transpose(1, 0, 2)
        ).reshape(P, DCHUNKS * D)
        in_maps.append({"xg": xg, "w": w_hw, "g": g_sb})
    return in_maps, Tmax, slot


def _build_program(Tmax):
    import concourse.tile as tile
    from concourse import bacc, mybir

    fp16 = mybir.dt.float16
    f32 = mybir.dt.float32

    nc = bacc.Bacc("TRN2", target_bir_lowering=False, debug=False)
    xg_d = nc.dram_tensor("xg", [P, Tmax * D], fp16, kind="ExternalInput").ap()
    w_d = nc.dram_tensor("w", [P, DCHUNKS * D], fp16, kind="ExternalInput").ap()
    g_d = nc.dram_tensor("g", [P, Tmax], f32, kind="ExternalInput").ap()
    y_d = nc.dram_tensor("y", [Tmax * P, D], fp16, kind="ExternalOutput").ap()

    with tile.TileContext(nc) as tc:
        with (
            tc.tile_pool(name="const", bufs=1) as cpool,
            tc.tile_pool(name="xp", bufs=Tmax) as xpool,
            tc.tile_pool(name="yp", bufs=3) as ypool,
            tc.tile_pool(name="warm", bufs=1, space="PSUM") as warmpool,
            tc.tile_pool(name="ps", bufs=3, space="PSUM") as pspool,
        ):
            # gates + x tiles on the scalar HWDGE ring
            g_sb = cpool.tile([P, Tmax], f32)
            nc.scalar.dma_start(g_sb[:], g_d[:])
            xgs = []
            for t in range(Tmax):
                xg = xpool.tile([P, D], fp16, tag="xg", name="xg")
                nc.scalar.dma_start(xg[:], xg_d[:, t * D : (t + 1) * D])
                xgs.append(xg)
            # W chunks on the sync ring (y stores go there too)
            wcs = []
            for c in range(DCHUNKS):
                w_sb = cpool.tile([P, D], fp16, name=f"w{c}")
                nc.sync.dma_start(w_sb[:], w_d[:, c * D : (c + 1) * D])
                wcs.append(w_sb)

            # Short PE warmup: covers the first xg/w DMA latency.
            ones = cpool.tile([1, P], fp16)
            nc.vector.memset(ones[:], 1.0)
            warm_ps = warmpool.tile([P, P], f32, tag="warm")
            for _ in range(WARMUP):
                nc.tensor.matmul(
                    warm_ps[:], ones[0:1, :], ones[0:1, :], start=True, stop=True
                )

            for t in range(Tmax):
                ps = pspool.tile([P, D], f32, tag="ps", name="ps")
                for c in range(DCHUNKS):
                    lhsT = xgs[t][:, c * P : (c + 1) * P]
                    for h in range(NH):
                        nc.tensor.matmul(
                            ps[:, h * FH : (h + 1) * FH],
                            lhsT,
                            wcs[c][:, h * FH : (h + 1) * FH],
                            start=(c == 0),
                            stop=(c == DCHUNKS - 1),
                        )
                y_sb = ypool.tile([P, D], fp16, tag="y", name="y_sb")
                nc.vector.tensor_scalar_mul(y_sb[:], ps[:], g_sb[:, t : t + 1])
                nc.sync.dma_start(y_d[t * P : (t + 1) * P, :], y_sb[:])
    nc.compile()
    return nc


def kernel(x, expert_weights, top_k_indices, W, b):
    from concourse.bass_utils import run_bass_kernel_spmd

    in_maps, Tmax, slot = _prep_inputs(x, top_k_indices, expert_weights, W)
    nc = _build_program(Tmax)
    res = run_bass_kernel_spmd(
        nc,
        in_maps,
        core_ids=list(range(N_CORES)),
        trace=bool(int(os.environ.get("KERNEL_TRACE", "0"))),
    )
    Y = np.concatenate(
        [res.results[e]["y"] for e in range(E)] + [np.zeros((1, D), np.float16)]
    ).astype(np.float32)
    idx = np.asarray(top_k_indices).reshape(-1, K)
    gw = np.asarray(expert_weights, np.float32).reshape(-1, K)
    b32 = np.asarray(b, np.float32)
    out = Y[slot[:, 0]] + Y[slot[:, 1]]
    out += gw[:, 0, None] * b32[idx[:, 0]]
    out += gw[:, 1, None] * b32[idx[:, 1]]
    if bool(int(os.environ.get("KERNEL_TRACE", "0"))):
        kernel.last_results = res
    return np.ascontiguousarray(out.reshape(B, S, D))


# revision 5
# speedup vs baseline: 1.6243x; 1.0001x over previous
"""MoE top-2 routing kernel for 8 Trainium2 NeuronCores — expert-parallel.

Problem: x[2,4096,1024] tokens, 8 experts W[8,1024,1024]+b[8,1024],
top-2 expert indices + gate weights per token.
out[t] = sum_k gate[t,k] * (x[t] @ W[idx[t,k]] + b[idx[t,k]])

Strategy (expert-parallel, host-side dispatch):
- E == n_cores == 8: core e owns expert e. The host routes: dedup the two
  (expert, gate) entries per token (same-expert duplicates merge, gates
  summed), groups entries by expert, and builds per-core inputs:
    xg   [128, Tmax*1024] fp16 — the expert's token rows, PE-transposed
         ([p, t*1024 + c*128 + m] = x[row t*128+m, c*128+p]), zero-padded
         to Tmax 128-row tiles.
    w    [128, 8*1024]    fp16 — W_e PE layout ([p, c*1024+f] = W_e[c*128+p, f])
    g    [128, Tmax]      f32  — gate per dispatch row ([m, t] = gate row t*128+m)
- Device: per 128-row tile, 16 accumulating fp16 matmuls (8 K-chunks x 2
  PSUM halves) -> DVE gate-scale (f32 PSUM -> fp16) -> contiguous DMA
  store. No gather/scatter ucode, no replicated W, no bias matmuls.
- Host combine: out[t] = Y[slot0[t]] + Y[slot1[t]] + g0*b[e0] + g1*b[e1]
  (slot1 -> zero row for merged/single-entry tokens); bias exact in f32.
- Load balance: per-expert entry counts are multinomial(~1920 +/- 40);
  Tmax = max_e ceil(n_e/128) == ceil(total_tiles/8) for typical draws, so
  expert-parallel matches the best possible row balance.
"""

import os
import sys

import numpy as np

for _p in ("/opt/trn_rl_repo", os.path.expanduser("~/.axon_site/_ro/trn_rl_repo")):
    if os.path.isdir(_p) and _p not in sys.path:
        sys.path.insert(0, _p)

B, S, D, E, K = 2, 4096, 1024, 8, 2
N_CORES = 8
TOKENS = B * S
P = 128
DCHUNKS = D // P  # 8
FH = 512  # psum bank half of D
NH = D // FH  # 2
WARMUP = 24


def _route(top_k_indices, expert_weights):
    """Dedup + group entries by expert.

    Returns (toks, gs, n_e, Tmax, cum, slot):
      toks/gs: token id and gate per dispatch entry, sorted by expert
      n_e[e]: entry count of expert e; cum[e]: its offset in the sort
      Tmax: per-core tile count = max_e ceil(n_e/128)
      slot[t, 0:2]: global padded-Y row of token t's entries (ZROW = none)
    """
    idx = np.asarray(top_k_indices).reshape(-1, K).astype(np.int64)
    gw = np.asarray(expert_weights).reshape(-1, K).astype(np.float32)
    dup = idx[:, 0] == idx[:, 1]
    g0 = np.where(dup, gw[:, 0] + gw[:, 1], gw[:, 0])
    keep = ~dup
    toks = np.concatenate([np.arange(TOKENS), np.arange(TOKENS)[keep]])
    exps = np.concatenate([idx[:, 0], idx[keep, 1]])
    gs = np.concatenate([g0, gw[keep, 1]])
    order = np.argsort(exps, kind="stable")
    toks, exps, gs = toks[order], exps[order], gs[order]
    n_e = np.bincount(exps, minlength=E)
    Tmax = max(1, int(np.max(-(-n_e // P))))
    cum = np.concatenate([[0], np.cumsum(n_e)])[:E]
    pos_in_e = np.arange(toks.size) - cum[exps]
    yrow = exps * (Tmax * P) + pos_in_e
    inv = np.empty_like(order)
    inv[order] = np.arange(order.size)
    ZROW = E * Tmax * P
    slot = np.full((TOKENS, 2), ZROW, np.int64)
    slot[:, 0] = yrow[inv[:TOKENS]]
    slot[keep, 1] = yrow[inv[TOKENS:]]
    return toks, gs, n_e, Tmax, cum, slot


def _prep_inputs(x, top_k_indices, expert_weights, W):
    toks, gs, n_e, Tmax, cum, slot = _route(top_k_indices, expert_weights)
    x_flat = np.asarray(x, np.float32).reshape(TOKENS, D).astype(np.float16)
    W16 = np.asarray(W, np.float32).astype(np.float16)
    in_maps = []
    for e in range(E):
        n = int(n_e[e])
        seg = slice(cum[e], cum[e] + n)
        xr = np.zeros((Tmax * P, D), np.float16)
        xr[:n] = x_flat[toks[seg]]
        # [t*128+m, c*128+p] -> [p, t, c, m]
        xg = np.ascontiguousarray(
            xr.reshape(Tmax, P, DCHUNKS, P).transpose(3, 0, 2, 1)
        ).reshape(P, Tmax * D)
        gr = np.zeros(Tmax * P, np.float32)
        gr[:n] = gs[seg]
        g_sb = np.ascontiguousarray(gr.reshape(Tmax, P).T)
        w_hw = np.ascontiguousarray(
            W16[e].reshape(DCHUNKS, P, D).transpose(1, 0, 2)
        ).reshape(P, DCHUNKS * D)
        in_maps.append({"xg": xg, "w": w_hw, "g": g_sb})
    return in_maps, Tmax, slot


def _build_program(Tmax):
    import concourse.tile as tile
    from concourse import bacc, mybir

    fp16 = mybir.dt.float16
    f32 = mybir.dt.float32

    nc = bacc.Bacc("TRN2", target_bir_lowering=False, debug=False)
    xg_d = nc.dram_tensor("xg", [P, Tmax * D], fp16, kind="ExternalInput").ap()
    w_d = nc.dram_tensor("w", [P, DCHUNKS * D], fp16, kind="ExternalInput").ap()
    g_d = nc.dram_tensor("g", [P, Tmax], f32, kind="ExternalInput").ap()
    y_d = nc.dram_tensor("y", [Tmax * P, D], fp16, kind="ExternalOutput").ap()

    with tile.TileContext(nc) as tc:
        with (
            tc.tile_pool(name="const", bufs=1) as cpool,
            tc.tile_pool(name="xp", bufs=Tmax) as xpool,
            tc.tile_pool(name="yp", bufs=3) as ypool,
            tc.tile_pool(name="warm", bufs=1, space="PSUM") as warmpool,
            tc.tile_pool(name="ps", bufs=3, space="PSUM") as pspool,
        ):
            # Two HWDGE queues (sync + scalar). First-needed data leads each:
            #   sync:   xg1, w0, w2, w4, w6, y-stores...
            #   scalar: xg0, w1, w3, w5, w7, g, xg2..xg15
            # so tiles 0-1 can start ~1 transfer after the prologue and ride
            # the interleaved W-chunk arrival stream without stalling.
            xgs = [xpool.tile([P, D], fp16, tag="xg", name="xg") for t in range(Tmax)]
            nc.sync.dma_start(xgs[1][:], xg_d[:, 1 * D : 2 * D])
            nc.scalar.dma_start(xgs[0][:], xg_d[:, 0:D])
            wcs = [cpool.tile([P, D], fp16, name=f"w{c}") for c in range(DCHUNKS)]
            for c in range(DCHUNKS):
                ring = nc.sync if c % 2 == 0 else nc.scalar
                ring.dma_start(wcs[c][:], w_d[:, c * D : (c + 1) * D])
            g_sb = cpool.tile([P, Tmax], f32)
            nc.scalar.dma_start(g_sb[:], g_d[:])
            for t in range(2, Tmax):
                nc.scalar.dma_start(xgs[t][:], xg_d[:, t * D : (t + 1) * D])

            # Short PE warmup: covers the first xg/w DMA latency + p-state ramp.
            ones = cpool.tile([1, P], fp16)
            nc.vector.memset(ones[:], 1.0)
            warm_ps = warmpool.tile([P, P], f32, tag="warm")
            for _ in range(WARMUP):
                nc.tensor.matmul(
                    warm_ps[:], ones[0:1, :], ones[0:1, :], start=True, stop=True
                )

            def scale_store(t, ps, split):
                """PSUM -> fp16 gate-scale -> DRAM store (halved when split to
                overlap the DVE pass with the store and use both queues)."""
                y_sb = ypool.tile([P, D], fp16, tag="y", name="y_sb")
                if not split:
                    nc.vector.tensor_scalar_mul(y_sb[:], ps[:], g_sb[:, t : t + 1])
                    nc.sync.dma_start(y_d[t * P : (t + 1) * P, :], y_sb[:])
                    return
                for h in range(NH):
                    sl = slice(h * FH, (h + 1) * FH)
                    nc.vector.tensor_scalar_mul(
                        y_sb[:, sl], ps[:, sl], g_sb[:, t : t + 1]
                    )
                    ring = nc.sync if h == 0 else nc.scalar
                    ring.dma_start(y_d[t * P : (t + 1) * P, sl], y_sb[:, sl])

            # Tiles 0-1 chunk-major: each W chunk is consumed by both tiles as
            # soon as it lands, so the PE tracks the W arrival stream.
            ps01 = [pspool.tile([P, D], f32, tag="ps", name="ps") for _ in range(2)]
            for c in range(DCHUNKS):
                for tt in range(2):
                    for h in range(NH):
                        nc.tensor.matmul(
                            ps01[tt][:, h * FH : (h + 1) * FH],
                            xgs[tt][:, c * P : (c + 1) * P],
                            wcs[c][:, h * FH : (h + 1) * FH],
                            start=(c == 0),
                            stop=(c == DCHUNKS - 1),
                        )
            for tt in range(2):
                scale_store(tt, ps01[tt], split=False)

            for t in range(2, Tmax):
                ps = pspool.tile([P, D], f32, tag="ps", name="ps")
                for c in range(DCHUNKS):
                    lhsT = xgs[t][:, c * P : (c + 1) * P]
                    for h in range(NH):
                        nc.tensor.matmul(
                            ps[:, h * FH : (h + 1) * FH],
                            lhsT,
                            wcs[c][:, h * FH : (h + 1) * FH],
                            start=(c == 0),
                            stop=(c == DCHUNKS - 1),
                        )
                scale_store(t, ps, split=(t == Tmax - 1))
    nc.compile()
    return nc


def kernel(x, expert_weights, top_k_indices, W, b):
    from concourse.bass_utils import run_bass_kernel_spmd

    in_maps, Tmax, slot = _prep_inputs(x, top_k_indices, expert_weights, W)
    nc = _build_program(Tmax)
    res = run_bass_kernel_spmd(
        nc,
        in_maps,
        core_ids=list(range(N_CORES)),
        trace=bool(int(os.environ.get("KERNEL_TRACE", "0"))),
    )
    Y = np.concatenate(
        [res.results[e]["y"] for e in range(E)] + [np.zeros((1, D), np.float16)]
    ).astype(np.float32)
    idx = np.asarray(top_k_indices).reshape(-1, K)
    gw = np.asarray(expert_weights, np.float32).reshape(-1, K)
    b32 = np.asarray(b, np.float32)
    out = Y[slot[:, 0]] + Y[slot[:, 1]]
    out += gw[:, 0, None] * b32[idx[:, 0]]
    out += gw[:, 1, None] * b32[idx[:, 1]]
    if bool(int(os.environ.get("KERNEL_TRACE", "0"))):
        kernel.last_results = res
    return np.ascontiguousarray(out.reshape(B, S, D))


# revision 8
# speedup vs baseline: 1.6650x; 1.0250x over previous
"""MoE top-2 routing kernel for 8 Trainium2 NeuronCores — expert-parallel.

Problem: x[2,4096,1024] tokens, 8 experts W[8,1024,1024]+b[8,1024],
top-2 expert indices + gate weights per token.
out[t] = sum_k gate[t,k] * (x[t] @ W[idx[t,k]] + b[idx[t,k]])

Strategy (expert-parallel, host-side dispatch):
- E == n_cores == 8: core e owns expert e. The host routes: dedup the two
  (expert, gate) entries per token (same-expert duplicates merge, gates
  summed), groups entries by expert, and builds per-core inputs:
    xg   [128, Tmax*1024] fp16 — the expert's token rows, PE-transposed
         ([p, t*1024 + c*128 + m] = x[row t*128+m, c*128+p]), zero-padded
         to Tmax 128-row tiles.
    w    [128, 8*1024]    fp16 — W_e PE layout ([p, c*1024+f] = W_e[c*128+p, f])
    g    [128, Tmax]      f32  — gate per dispatch row ([m, t] = gate row t*128+m)
- Device: per 128-row tile, 16 accumulating fp16 matmuls (8 K-chunks x 2
  PSUM halves) -> DVE gate-scale (f32 PSUM -> fp16) -> contiguous DMA
  store. No gather/scatter ucode, no replicated W, no bias matmuls.
- Host combine: out[t] = Y[slot0[t]] + Y[slot1[t]] + g0*b[e0] + g1*b[e1]
  (slot1 -> zero row for merged/single-entry tokens); bias exact in f32.
- Load balance: per-expert entry counts are multinomial(~1920 +/- 40);
  Tmax = max_e ceil(n_e/128) == ceil(total_tiles/8) for typical draws, so
  expert-parallel matches the best possible row balance.
"""

import os
import sys

import numpy as np

for _p in ("/opt/trn_rl_repo", os.path.expanduser("~/.axon_site/_ro/trn_rl_repo")):
    if os.path.isdir(_p) and _p not in sys.path:
        sys.path.insert(0, _p)

B, S, D, E, K = 2, 4096, 1024, 8, 2
N_CORES = 8
TOKENS = B * S
P = 128
DCHUNKS = D // P  # 8
FH = 512  # psum bank half of D
NH = D // FH  # 2
WARMUP = 16


def _route(top_k_indices, expert_weights):
    """Dedup + group entries by expert.

    Returns (toks, gs, n_e, Tmax, cum, slot):
      toks/gs: token id and gate per dispatch entry, sorted by expert
      n_e[e]: entry count of expert e; cum[e]: its offset in the sort
      Tmax: per-core tile count = max_e ceil(n_e/128)
      slot[t, 0:2]: global padded-Y row of token t's entries (ZROW = none)
    """
    idx = np.asarray(top_k_indices).reshape(-1, K).astype(np.int64)
    gw = np.asarray(expert_weights).reshape(-1, K).astype(np.float32)
    dup = idx[:, 0] == idx[:, 1]
    g0 = np.where(dup, gw[:, 0] + gw[:, 1], gw[:, 0])
    keep = ~dup
    toks = np.concatenate([np.arange(TOKENS), np.arange(TOKENS)[keep]])
    exps = np.concatenate([idx[:, 0], idx[keep, 1]])
    gs = np.concatenate([g0, gw[keep, 1]])
    order = np.argsort(exps, kind="stable")
    toks, exps, gs = toks[order], exps[order], gs[order]
    n_e = np.bincount(exps, minlength=E)
    Tmax = max(1, int(np.max(-(-n_e // P))))
    cum = np.concatenate([[0], np.cumsum(n_e)])[:E]
    pos_in_e = np.arange(toks.size) - cum[exps]
    yrow = exps * (Tmax * P) + pos_in_e
    inv = np.empty_like(order)
    inv[order] = np.arange(order.size)
    ZROW = E * Tmax * P
    slot = np.full((TOKENS, 2), ZROW, np.int64)
    slot[:, 0] = yrow[inv[:TOKENS]]
    slot[keep, 1] = yrow[inv[TOKENS:]]
    return toks, gs, n_e, Tmax, cum, slot


def _prep_inputs(x, top_k_indices, expert_weights, W):
    toks, gs, n_e, Tmax, cum, slot = _route(top_k_indices, expert_weights)
    x_flat = np.asarray(x, np.float32).reshape(TOKENS, D).astype(np.float16)
    W16 = np.asarray(W, np.float32).astype(np.float16)
    in_maps = []
    for e in range(E):
        n = int(n_e[e])
        seg = slice(cum[e], cum[e] + n)
        xr = np.zeros((Tmax * P, D), np.float16)
        xr[:n] = x_flat[toks[seg]]
        # [t*128+m, c*128+p] -> [p, t, c, m]
        xg = np.ascontiguousarray(
            xr.reshape(Tmax, P, DCHUNKS, P).transpose(3, 0, 2, 1)
        ).reshape(P, Tmax * D)
        gr = np.zeros(Tmax * P, np.float32)
        gr[:n] = gs[seg]
        g_sb = np.ascontiguousarray(gr.reshape(Tmax, P).T)
        w_hw = np.ascontiguousarray(
            W16[e].reshape(DCHUNKS, P, D).transpose(1, 0, 2)
        ).reshape(P, DCHUNKS * D)
        in_maps.append({"xg": xg, "w": w_hw, "g": g_sb})
    return in_maps, Tmax, slot


def _build_program(Tmax):
    import concourse.tile as tile
    from concourse import bacc, mybir

    fp16 = mybir.dt.float16
    f32 = mybir.dt.float32

    nc = bacc.Bacc("TRN2", target_bir_lowering=False, debug=False)
    xg_d = nc.dram_tensor("xg", [P, Tmax * D], fp16, kind="ExternalInput").ap()
    w_d = nc.dram_tensor("w", [P, DCHUNKS * D], fp16, kind="ExternalInput").ap()
    g_d = nc.dram_tensor("g", [P, Tmax], f32, kind="ExternalInput").ap()
    y_d = nc.dram_tensor("y", [Tmax * P, D], fp16, kind="ExternalOutput").ap()

    with tile.TileContext(nc) as tc:
        with (
            tc.tile_pool(name="const", bufs=1) as cpool,
            tc.tile_pool(name="xp", bufs=Tmax) as xpool,
            tc.tile_pool(name="yp", bufs=3) as ypool,
            tc.tile_pool(name="warm", bufs=1, space="PSUM") as warmpool,
            tc.tile_pool(name="ps", bufs=3, space="PSUM") as pspool,
        ):
            # Two HWDGE queues (sync SP + scalar ACT), each ~130 GB/s when both
            # stream and limited to 4 in-flight transfers. The head is
            # bandwidth-bound: W (2 MiB) + xg0/xg1 must land before tiles 0-1
            # finish, so supply is issued in half-chunk (128 KB) transfers,
            # dealt across the queues in consumption order; tiles 0-1 are
            # computed chunk-major so the PE rides the arrival stream.
            xgs = [xpool.tile([P, D], fp16, tag="xg", name="xg") for t in range(Tmax)]
            whs = [
                [cpool.tile([P, FH], fp16, name=f"w{c}h{h}") for h in range(NH)]
                for c in range(DCHUNKS)
            ]
            g_sb = cpool.tile([P, Tmax], f32)

            def kick_xg_half(ring, t, a):
                ring.dma_start(
                    xgs[t][:, a * FH : (a + 1) * FH],
                    xg_d[:, t * D + a * FH : t * D + (a + 1) * FH],
                )

            def kick_w(ring, c, h):
                ring.dma_start(
                    whs[c][h][:], w_d[:, c * D + h * FH : c * D + (h + 1) * FH]
                )

            # Supply deal-out in need order (tiles 0-1 chunk-major consumption).
            kick_xg_half(nc.sync, 0, 0)
            kick_w(nc.scalar, 0, 0)
            kick_xg_half(nc.sync, 1, 0)
            kick_w(nc.scalar, 0, 1)
            kick_w(nc.sync, 1, 0)
            kick_w(nc.scalar, 1, 1)
            kick_w(nc.sync, 2, 0)
            kick_w(nc.scalar, 2, 1)
            kick_w(nc.sync, 3, 0)
            kick_w(nc.scalar, 3, 1)
            kick_xg_half(nc.sync, 0, 1)
            kick_w(nc.scalar, 4, 0)
            kick_xg_half(nc.sync, 1, 1)
            kick_w(nc.scalar, 4, 1)
            kick_w(nc.sync, 5, 0)
            kick_w(nc.scalar, 5, 1)
            kick_w(nc.sync, 6, 0)
            kick_w(nc.scalar, 6, 1)
            kick_w(nc.sync, 7, 0)
            kick_w(nc.scalar, 7, 1)
            nc.scalar.dma_start(g_sb[:], g_d[:])
            for t in range(2, Tmax):
                nc.scalar.dma_start(xgs[t][:], xg_d[:, t * D : (t + 1) * D])

            # Short PE warmup: covers the first xg/w DMA latency + p-state ramp.
            ones = cpool.tile([1, P], fp16)
            nc.vector.memset(ones[:], 1.0)
            warm_ps = warmpool.tile([P, P], f32, tag="warm")
            for _ in range(WARMUP):
                nc.tensor.matmul(
                    warm_ps[:], ones[0:1, :], ones[0:1, :], start=True, stop=True
                )

            def scale_store(t, ps, split):
                """PSUM -> fp16 gate-scale -> DRAM store (halved when split to
                overlap the DVE pass with the store and use both queues)."""
                y_sb = ypool.tile([P, D], fp16, tag="y", name="y_sb")
                if not split:
                    nc.vector.tensor_scalar_mul(y_sb[:], ps[:], g_sb[:, t : t + 1])
                    nc.sync.dma_start(y_d[t * P : (t + 1) * P, :], y_sb[:])
                    return
                for h in range(NH):
                    sl = slice(h * FH, (h + 1) * FH)
                    nc.vector.tensor_scalar_mul(
                        y_sb[:, sl], ps[:, sl], g_sb[:, t : t + 1]
                    )
                    ring = nc.sync if h == 0 else nc.scalar
                    ring.dma_start(y_d[t * P : (t + 1) * P, sl], y_sb[:, sl])

            # Tiles 0-1 chunk-major: each W half-chunk is consumed by both
            # tiles as soon as it lands, so the PE tracks the arrival stream.
            ps01 = [pspool.tile([P, D], f32, tag="ps", name="ps") for _ in range(2)]
            for c in range(DCHUNKS):
                for tt in range(2):
                    for h in range(NH):
                        nc.tensor.matmul(
                            ps01[tt][:, h * FH : (h + 1) * FH],
                            xgs[tt][:, c * P : (c + 1) * P],
                            whs[c][h][:],
                            start=(c == 0),
                            stop=(c == DCHUNKS - 1),
                        )
            for tt in range(2):
                scale_store(tt, ps01[tt], split=False)

            for t in range(2, Tmax):
                ps = pspool.tile([P, D], f32, tag="ps", name="ps")
                for c in range(DCHUNKS):
                    lhsT = xgs[t][:, c * P : (c + 1) * P]
                    for h in range(NH):
                        nc.tensor.matmul(
                            ps[:, h * FH : (h + 1) * FH],
                            lhsT,
                            whs[c][h][:],
                            start=(c == 0),
                            stop=(c == DCHUNKS - 1),
                        )
                scale_store(t, ps, split=(t >= Tmax - 2))
    nc.compile()
    return nc


def kernel(x, expert_weights, top_k_indices, W, b):
    from concourse.bass_utils import run_bass_kernel_spmd

    in_maps, Tmax, slot = _prep_inputs(x, top_k_indices, expert_weights, W)
    nc = _build_program(Tmax)
    res = run_bass_kernel_spmd(
        nc,
        in_maps,
        core_ids=list(range(N_CORES)),
        trace=bool(int(os.environ.get("KERNEL_TRACE", "0"))),
    )
    Y = np.concatenate(
        [res.results[e]["y"] for e in range(E)] + [np.zeros((1, D), np.float16)]
    ).astype(np.float32)
    idx = np.asarray(top_k_indices).reshape(-1, K)
    gw = np.asarray(expert_weights, np.float32).reshape(-1, K)
    b32 = np.asarray(b, np.float32)
    out = Y[slot[:, 0]] + Y[slot[:, 1]]
    out += gw[:, 0, None] * b32[idx[:, 0]]
    out += gw[:, 1, None] * b32[idx[:, 1]]
    if bool(int(os.environ.get("KERNEL_TRACE", "0"))):
        kernel.last_results = res
    return np.ascontiguousarray(out.reshape(B, S, D))
